# revision 12
# baseline (speedup 1.0000x reference)
"""Trainium2 Bass kernel for nn_BUNet (GCN mol+pro branches, PPI branch, head).

Self-contained: host graph preprocessing + SPMD Bass/Tile program on 8
NeuronCores + output assembly.  Sharding: graph-aligned node shards per core,
edges partitioned by destination; per-layer fp16 AllGather of hidden tables;
PPI branch replicated with dense normalized adjacency; head sharded by pair.
"""
import sys
sys.path.insert(0, '/opt/trn_rl_repo')
import numpy as np

from concourse import bass, mybir
import concourse.bacc as bacc
import concourse.tile as tile
from concourse.masks import make_identity
from concourse.tile import add_dep_helper

NCORES = 8
SLAB = 256        # chunks of edge-metadata per streaming DMA
f16 = mybir.dt.float16
f32 = mybir.dt.float32
i32 = mybir.dt.int32
RELU = mybir.ActivationFunctionType.Relu
IDENT = mybir.ActivationFunctionType.Identity

_CACHE = {}


# ----------------------------------------------------------------------------
# Embedded SPMD runner (PJRT path, persistent jit)
# ----------------------------------------------------------------------------

class SpmdRunner:
    def __init__(self, nc, n_cores):
        import jax
        from jax.sharding import Mesh, PartitionSpec
        from jax.experimental.shard_map import shard_map
        from concourse.bass2jax import (_bass_exec_p, install_neuronx_cc_hook,
                                        partition_id_tensor)
        self.jax = jax
        install_neuronx_cc_hook()
        self.nc = nc
        self.n_cores = n_cores
        partition_name = (nc.partition_id_tensor.name
                          if nc.partition_id_tensor else None)
        in_names, out_names, out_avals, zero_outs = [], [], [], []
        for alloc in nc.m.functions[0].allocations:
            if not isinstance(alloc, mybir.MemoryLocationSet):
                continue
            name = alloc.memorylocations[0].name
            if alloc.kind == "ExternalInput":
                if name != partition_name:
                    in_names.append(name)
            elif alloc.kind == "ExternalOutput":
                out_names.append(name)
                shape = tuple(alloc.tensor_shape)
                dtype = mybir.dt.np(alloc.dtype)
                out_avals.append(jax.core.ShapedArray(shape, dtype))
                zero_outs.append(np.zeros(shape, dtype))
        self.in_names = list(in_names)
        self.out_names = out_names
        self.out_avals = out_avals
        self.zero_outs = zero_outs
        n_params = len(self.in_names)
        n_outs = len(out_names)
        all_in_names = self.in_names + out_names
        if partition_name is not None:
            all_in_names.append(partition_name)

        def _body(*args):
            operands = list(args)
            if partition_name is not None:
                operands.append(partition_id_tensor())
            outs = _bass_exec_p.bind(
                *operands, out_avals=tuple(out_avals),
                in_names=tuple(all_in_names), out_names=tuple(out_names),
                lowering_input_output_aliases=(), sim_require_finite=True,
                sim_require_nnan=True, nc=nc)
            return tuple(outs)

        devices = jax.devices()[:n_cores]
        self.mesh = Mesh(np.asarray(devices), ("core",))
        in_specs = (PartitionSpec("core"),) * (n_params + n_outs)
        out_specs = (PartitionSpec("core"),) * n_outs
        donate = tuple(range(n_params, n_params + n_outs))
        self.fn = jax.jit(
            shard_map(_body, mesh=self.mesh, in_specs=in_specs,
                      out_specs=out_specs, check_rep=False),
            donate_argnums=donate, keep_unused=True)
        self.resident = None

    def put_inputs(self, in_maps):
        from jax.sharding import NamedSharding, PartitionSpec
        concat = [
            np.concatenate([np.asarray(in_maps[c][n])
                            for c in range(self.n_cores)], axis=0)
            for n in self.in_names]
        sh = NamedSharding(self.mesh, PartitionSpec("core"))
        self.resident = [self.jax.device_put(a, sh) for a in concat]

    def run(self):
        zeros = [np.zeros((self.n_cores * z.shape[0], *z.shape[1:]), z.dtype)
                 for z in self.zero_outs]
        out = self.fn(*self.resident, *zeros)
        self.jax.block_until_ready(out)
        return out

    def results(self, outs):
        res = []
        for c in range(self.n_cores):
            d = {}
            for i, name in enumerate(self.out_names):
                d[name] = np.asarray(outs[i]).reshape(
                    self.n_cores, *self.out_avals[i].shape)[c]
            res.append(d)
        return res


# ----------------------------------------------------------------------------
# Host preprocessing
# ----------------------------------------------------------------------------

def _prep_graph(edge_index, batch, n_graphs):
    """Graph-aligned node sharding + dst-partitioned chunked edges."""
    n = batch.shape[0]
    src = edge_index[0].astype(np.int64)
    dst = edge_index[1].astype(np.int64)
    loops = np.arange(n, dtype=np.int64)
    src_a = np.concatenate([src, loops])
    dst_a = np.concatenate([dst, loops])
    deg = np.bincount(dst_a, minlength=n).astype(np.float64)
    dis = 1.0 / np.sqrt(np.maximum(deg, 1.0))
    dis[deg <= 0] = 0.0
    coeff = (dis[src_a] * dis[dst_a]).astype(np.float32)

    gpc = n_graphs // NCORES
    node_core = (batch.astype(np.int64) // gpc).clip(0, NCORES - 1)
    shard_start = np.searchsorted(node_core, np.arange(NCORES))
    shard_end = np.searchsorted(node_core, np.arange(NCORES), side='right')
    shard_size = shard_end - shard_start
    s_max = int(np.ceil(max(shard_size.max(), 1) / 128) * 128)
    ntiles = s_max // 128
    np_tot = NCORES * s_max

    pid = np.empty(n, dtype=np.int64)
    for r in range(NCORES):
        sl = slice(shard_start[r], shard_end[r])
        pid[sl] = r * s_max + np.arange(shard_size[r])

    src_p = pid[src_a]
    dst_p = pid[dst_a]
    e_core = dst_p // s_max
    dst_tile_local = (dst_p % s_max) // 128

    cnt = np.zeros((NCORES, ntiles), dtype=np.int64)
    order_all = []
    for r in range(NCORES):
        m = np.nonzero(e_core == r)[0]
        o = m[np.argsort(dst_p[m], kind='stable')]
        order_all.append(o)
        c = np.bincount(dst_tile_local[o], minlength=ntiles)
        cnt[r] = np.ceil(c / 128).astype(np.int64)
    cnt_g = np.maximum(cnt.max(axis=0), 1)
    cnt_g[-1] += (-int(cnt_g.sum())) % 16
    c_tot = int(cnt_g.sum())
    tile_ofs = np.concatenate([[0], np.cumsum(cnt_g)])

    srcs, dstls, coes = [], [], []
    for r in range(NCORES):
        o = order_all[r]
        t = dst_tile_local[o]
        sarr = np.zeros((c_tot * 128,), dtype=np.int32)
        darr = np.zeros((c_tot * 128,), dtype=np.float32)
        carr = np.zeros((c_tot * 128,), dtype=np.float32)
        tile_cnt = np.bincount(t, minlength=ntiles)
        pos = 0
        for ti in range(ntiles):
            k = int(tile_cnt[ti])
            sel = o[pos:pos + k]
            pos += k
            base = int(tile_ofs[ti]) * 128
            sarr[base:base + k] = src_p[sel].astype(np.int32)
            darr[base:base + k] = (dst_p[sel] % 128).astype(np.float32)
            carr[base:base + k] = coeff[sel]
        srcs.append(np.ascontiguousarray(sarr.reshape(c_tot, 128).T))
        dstls.append(np.ascontiguousarray(darr.reshape(c_tot, 128).T))
        coes.append(np.ascontiguousarray(carr.reshape(c_tot, 128).T))

    gcnt = np.bincount(batch.astype(np.int64), minlength=n_graphs).astype(np.float64)
    inv = np.where(gcnt > 0, 1.0 / np.maximum(gcnt, 1.0), 0.0)
    bls, ics = [], []
    for r in range(NCORES):
        bl = np.zeros((s_max,), dtype=np.float32)
        ic = np.zeros((s_max,), dtype=np.float32)
        sl = slice(shard_start[r], shard_end[r])
        sz = int(shard_size[r])
        bidx = batch[sl].astype(np.int64)
        bl[:sz] = (bidx - r * gpc).astype(np.float32)
        ic[:sz] = inv[bidx].astype(np.float32)
        bls.append(np.ascontiguousarray(bl.reshape(ntiles, 128).T))
        ics.append(np.ascontiguousarray(ic.reshape(ntiles, 128).T))

    return dict(gpc=gpc, s_max=s_max, ntiles=ntiles, np_tot=np_tot,
                cnt_g=cnt_g, c_tot=c_tot, pid=pid,
                src=srcs, dstl=dstls, coe=coes, bl=bls, ic=ics)


def _pad_x(x, pid, np_tot):
    out = np.zeros((np_tot, x.shape[1]), dtype=np.float16)
    out[pid] = x.astype(np.float16)
    return out


def _prep_ppi(ppi_edge, b_pro, gpc, gp_pad):
    gp_tot = NCORES * gp_pad
    qs = ppi_edge[0].astype(np.int64)
    qd = ppi_edge[1].astype(np.int64)
    deg = np.bincount(qd, minlength=b_pro) + 1.0
    dis = 1.0 / np.sqrt(deg)

    def pg(g):
        return (g // gpc) * gp_pad + (g % gpc)

    A = np.zeros((gp_tot, gp_tot), dtype=np.float32)
    np.add.at(A, (pg(qd), pg(qs)), (dis[qd] * dis[qs]).astype(np.float32))
    gids = np.arange(b_pro, dtype=np.int64)
    A[pg(gids), pg(gids)] += (dis * dis).astype(np.float32)
    return np.ascontiguousarray(A.T).astype(np.float16), pg


# ----------------------------------------------------------------------------
# Device program
# ----------------------------------------------------------------------------

DIMS = dict(pg1=(33, 128), pg2=(128, 128), pg3=(128, 128),
            mg1=(78, 156), mg2=(156, 312), mg3=(312, 128),
            pfc1=(128, 1024), pfc2=(1024, 128),
            mfc1=(128, 1024), mfc2=(1024, 128),
            ppig1=(128, 1024), ppig2=(1024, 128),
            ppifc1=(128, 1024), ppifc2=(1024, 128),
            fc1=(256, 1024), fc2=(1024, 512), out=(512, 1))

WMAP = dict(pg1="w_pg1", pg2="w_pg2", pg3="w_pg3", mg1="w_mg1", mg2="w_mg2",
            mg3="w_mg3", pfc1="w_pfc1", pfc2="w_pfc2", mfc1="w_mfc1",
            mfc2="w_mfc2", ppig1="w_ppig1", ppig2="w_ppig2",
            ppifc1="w_ppifc1", ppifc2="w_ppifc2", fc1="w_fc1", fc2="w_fc2",
            out="w_out")
BMAP = {k: "b" + v[1:] for k, v in WMAP.items()}
REPL_BIAS = {"pg1", "pg2", "pg3", "mg1", "mg2", "mg3"}


def _bias_host(name, b):
    b = np.asarray(b)
    if name in REPL_BIAS:
        return np.tile(b.astype(np.float32)[None, :], (128, 1))
    n = b.shape[0]
    if n % 128 == 0:
        return np.ascontiguousarray(b.astype(np.float32).reshape(-1, 128).T)
    assert n == 1
    return b.astype(np.float32).reshape(1, 1)


def _kg_for(ch):
    # HW indirect DMA consumes one index per partition (contiguous free dim),
    # so multi-chunk gathers are incorrect on hardware; keep one chunk per DMA.
    return 1


def _build_program(meta):
    mp, mm = meta['pro'], meta['mol']
    gp_pad = meta['gp_pad']
    gm_pc = meta['gm_pc']
    gp_tot = NCORES * gp_pad
    ntok = gp_tot // 128
    nsl = gp_tot // 512

    nc = bacc.Bacc(None, target_bir_lowering=False, debug=False)

    def par(name, shape, dt):
        return nc.declare_dram_parameter(name, list(shape), dt, isOutput=False)

    x_pro = par("x_pro", (mp['np_tot'], 33), f16)
    x_mol = par("x_mol", (mm['np_tot'], 78), f16)
    e_in = {}
    for b, m in (("p", mp), ("m", mm)):
        e_in[b + "idx"] = par(b + "_idx", (128, m['c_tot']), i32)
        e_in[b + "dst"] = par(b + "_dst", (128, m['c_tot']), f32)
        e_in[b + "co"] = par(b + "_co", (128, m['c_tot']), f32)
        e_in[b + "bl"] = par(b + "_bl", (128, m['ntiles']), f32)
        e_in[b + "ic"] = par(b + "_ic", (128, m['ntiles']), f32)
    iota_in = par("iota", (128, 512), f16)
    at_in = par("at", (gp_tot, gp_tot), f16)
    seq_in = par("seq", (128, gm_pc // 128), i32)
    w_in, wb_in = {}, {}
    for n, (a, b) in DIMS.items():
        w_in[n] = par("w_" + n, (a, b), f16)
        wb_in[n] = par("b_" + n, meta['bias_shape'][n], f32)
    out_par = nc.declare_dram_parameter("out", [1, gm_pc], f32, isOutput=True)

    h_p1_in = nc.dram_tensor("hp1i", [mp['s_max'], 128], f16)
    h_p2_in = nc.dram_tensor("hp2i", [mp['s_max'], 128], f16)
    h_m1_in = nc.dram_tensor("hm1i", [mm['s_max'], 156], f16)
    h_m2_in = nc.dram_tensor("hm2i", [mm['s_max'], 312], f16)
    h_p1 = nc.dram_tensor("hp1", [mp['np_tot'], 128], f16, addr_space="Shared")
    h_p2 = nc.dram_tensor("hp2", [mp['np_tot'], 128], f16, addr_space="Shared")
    h_m1 = nc.dram_tensor("hm1", [mm['np_tot'], 156], f16, addr_space="Shared")
    h_m2 = nc.dram_tensor("hm2", [mm['np_tot'], 312], f16, addr_space="Shared")
    p_ag_in = nc.dram_tensor("pagi", [gp_pad, 128], f32)
    p_full = nc.dram_tensor("pfull", [gp_tot, 128], f32, addr_space="Shared")
    q_rows = nc.dram_tensor("qrows", [gp_tot, 128], f32)
    RG = [list(range(NCORES))]

    with tile.TileContext(nc, num_cores=NCORES) as tc:
        with (
            tc.tile_pool(name="const", bufs=1) as cpool,
            tc.tile_pool(name="meta", bufs=2) as ipool,
            tc.tile_pool(name="g", bufs=2) as gpool,
            tc.tile_pool(name="s", bufs=6) as spool,
            tc.tile_pool(name="aggs", bufs=3) as apool,
            tc.tile_pool(name="h", bufs=3) as hpool,
            tc.tile_pool(name="big", bufs=1) as bpool,
            tc.tile_pool(name="at", bufs=18) as atpool,
            tc.tile_pool(name="psA", bufs=3, space="PSUM") as psA,
            tc.tile_pool(name="psB", bufs=3, space="PSUM") as psB,
            tc.tile_pool(name="psPool", bufs=1, space="PSUM") as psP,
        ):
            # ---------------- constants ----------------
            iota = cpool.tile([128, 512], f16)
            nc.sync.dma_start(iota[:], iota_in[:])
            ident = cpool.tile([128, 128], f32)
            make_identity(nc, ident[:])
            W, B = {}, {}
            for n, (a, b) in DIMS.items():
                tiles = []
                for j in range((a + 127) // 128):
                    aj = min(128, a - j * 128)
                    t = cpool.tile([aj, b], f16, tag=f"w{n}{j}")
                    nc.sync.dma_start(t[:], w_in[n][j * 128:j * 128 + aj, :])
                    tiles.append(t)
                W[n] = tiles
                shp = meta['bias_shape'][n]
                bt = cpool.tile(list(shp), f32, tag=f"b{n}")
                nc.sync.dma_start(bt[:], wb_in[n][:])
                B[n] = bt
            bl_p = cpool.tile([128, mp['ntiles']], f32)
            ic_p = cpool.tile([128, mp['ntiles']], f32)
            bl_m = cpool.tile([128, mm['ntiles']], f32)
            ic_m = cpool.tile([128, mm['ntiles']], f32)
            nc.sync.dma_start(bl_p[:], e_in["pbl"][:])
            nc.sync.dma_start(ic_p[:], e_in["pic"][:])
            nc.sync.dma_start(bl_m[:], e_in["mbl"][:])
            nc.sync.dma_start(ic_m[:], e_in["mic"][:])
            seq_t = cpool.tile([128, gm_pc // 128], i32)
            nc.sync.dma_start(seq_t[:], seq_in[:])

            pool_p = psP.tile([128, gp_pad], f32, tag="poolP")
            pool_m = psP.tile([128, gm_pc], f32, tag="poolM")

            # ---------------- GCN layer ----------------
            def gcn(branch, lname, ch, feat, table, wn, ag_in=None,
                    pool_target=None, pool_bl=None, pool_ic=None, pool_w=0):
                m = mp if branch == "p" else mm
                cnt_g, ntiles, c_tot = m['cnt_g'], m['ntiles'], m['c_tot']
                idx_in, dst_in, co_in = (e_in[branch + "idx"],
                                         e_in[branch + "dst"],
                                         e_in[branch + "co"])
                kg = _kg_for(ch)
                nch = (ch + 127) // 128
                aggw = 128 * nch
                tile_of_chunk = np.repeat(np.arange(ntiles), cnt_g)
                tile_ofs = np.concatenate([[0], np.cumsum(cnt_g)])
                it = dt_ = ct = None
                gt, gbase = None, 0
                agg_p = None
                for c in range(c_tot):
                    if c % SLAB == 0:
                        w = min(SLAB, c_tot - c)
                        it = ipool.tile([128, SLAB], i32, tag="eidx")
                        dt_ = ipool.tile([128, SLAB], f32, tag="edst")
                        ct = ipool.tile([128, SLAB], f32, tag="eco")
                        nc.sync.dma_start(it[:, :w], idx_in[:, c:c + w])
                        nc.sync.dma_start(dt_[:, :w], dst_in[:, c:c + w])
                        nc.sync.dma_start(ct[:, :w], co_in[:, c:c + w])
                    if c % kg == 0:
                        gt = gpool.tile([128, 2624], f16, tag="g")
                        so = c % SLAB
                        nc.gpsimd.indirect_dma_start(
                            out=gt[:, :kg * ch], out_offset=None, in_=table[:],
                            in_offset=bass.IndirectOffsetOnAxis(
                                ap=it[:, so:so + kg], axis=0))
                        tail = (128 - ch % 128) % 128
                        if tail:
                            nc.vector.memset(
                                gt[:, kg * ch:kg * ch + tail], 0.0)
                        gbase = c
                    t = int(tile_of_chunk[c])
                    first = (c == tile_ofs[t])
                    last = (c == tile_ofs[t + 1] - 1)
                    if first:
                        agg_p = psA.tile([128, 384], f32, tag="agg")
                    st = spool.tile([128, 256], f16, tag="s")
                    so = c % SLAB
                    nc.vector.tensor_scalar(
                        out=st[:, :128], in0=iota[:, :128],
                        scalar1=dt_[:, so:so + 1], scalar2=ct[:, so:so + 1],
                        op0=mybir.AluOpType.is_equal, op1=mybir.AluOpType.mult)
                    co = (c - gbase) * ch
                    for j in range(nch):
                        nc.tensor.matmul(
                            agg_p[:, j * 128:j * 128 + 128],
                            gt[:, co + j * 128: co + j * 128 + 128],
                            st[:, :128], start=(first and j == 0),
                            stop=(last and j == nch - 1))
                    if last:
                        agg_s = apool.tile([128, 384], f16, tag="aggs")
                        nc.vector.tensor_copy(agg_s[:, :aggw], agg_p[:, :aggw])
                        out_p = psB.tile([128, 512], f32, tag="ps")
                        for j in range(nch):
                            chj = min(128, ch - j * 128)
                            nc.tensor.matmul(
                                out_p[:, :feat],
                                agg_s[0:chj, j * 128:j * 128 + 128],
                                W[wn][j][:], start=(j == 0), stop=(j == nch - 1))
                        h_t = hpool.tile([128, 384], f16, tag="gh")
                        nc.vector.tensor_tensor(
                            out=h_t[:, :feat], in0=out_p[:, :feat],
                            in1=B[wn][:, :feat], op=mybir.AluOpType.add)
                        nc.scalar.activation(h_t[:, :feat], h_t[:, :feat], RELU)
                        if ag_in is not None:
                            nc.sync.dma_start(
                                ag_in[t * 128:(t + 1) * 128, :], h_t[:, :feat])
                        if pool_target is not None:
                            sp = spool.tile([128, 256], f16, tag="s")
                            nc.vector.tensor_scalar(
                                out=sp[:, :pool_w], in0=iota[:, :pool_w],
                                scalar1=pool_bl[:, t:t + 1],
                                scalar2=pool_ic[:, t:t + 1],
                                op0=mybir.AluOpType.is_equal,
                                op1=mybir.AluOpType.mult)
                            nc.tensor.matmul(
                                pool_target[:], h_t[:, :feat], sp[:, :pool_w],
                                start=(t == 0), stop=(t == ntiles - 1))

            def ag(src_t, dst_t):
                nc.gpsimd.collective_compute(
                    "AllGather", mybir.AluOpType.bypass, replica_groups=RG,
                    ins=[src_t[:]], outs=[dst_t[:]])

            # ---------------- schedule ----------------
            gcn("p", "pL1", 33, 128, x_pro, "pg1", ag_in=h_p1_in)
            ag(h_p1_in, h_p1)
            gcn("m", "mL1", 78, 156, x_mol, "mg1", ag_in=h_m1_in)
            ag(h_m1_in, h_m1)
            gcn("p", "pL2", 128, 128, h_p1, "pg2", ag_in=h_p2_in)
            ag(h_p2_in, h_p2)
            gcn("m", "mL2", 156, 312, h_m1, "mg2", ag_in=h_m2_in)
            ag(h_m2_in, h_m2)
            gcn("p", "pL3", 128, 128, h_p2, "pg3", pool_target=pool_p,
                pool_bl=bl_p, pool_ic=ic_p, pool_w=gp_pad)
            gcn("m", "mL3", 312, 128, h_m2, "mg3", pool_target=pool_m,
                pool_bl=bl_m, pool_ic=ic_m, pool_w=gm_pc)

            # ---------------- FC stacks (feature-major) ----------------
            def fc_stack(poolt, w1n, w2n, width):
                p1 = []
                for mch in range(8):
                    ps = psB.tile([128, 512], f32, tag="ps")
                    nc.tensor.matmul(ps[:, :width],
                                     W[w1n][0][:, mch * 128:(mch + 1) * 128],
                                     poolt[:], start=True, stop=True)
                    t = bpool.tile([128, 512], f16, tag=f"fcs{mch}")
                    nc.scalar.activation(t[:, :width], ps[:, :width], RELU,
                                         bias=B[w1n][:, mch:mch + 1])
                    p1.append(t)
                ps = psB.tile([128, 512], f32, tag="ps")
                for kch in range(8):
                    nc.tensor.matmul(ps[:, :width], W[w2n][kch][:],
                                     p1[kch][:, :width],
                                     start=(kch == 0), stop=(kch == 7))
                t = hpool.tile([128, 512], f32, tag="fco")
                nc.vector.tensor_scalar(out=t[:, :width], in0=ps[:, :width],
                                        scalar1=B[w2n][:, 0:1], scalar2=None,
                                        op0=mybir.AluOpType.add)
                return t

            poolp_s = bpool.tile([128, gp_pad], f16, tag="poolps")
            nc.vector.tensor_copy(poolp_s[:], pool_p[:])
            poolm_s = bpool.tile([128, gm_pc], f16, tag="poolms")
            nc.vector.tensor_copy(poolm_s[:], pool_m[:])
            pT = fc_stack(poolp_s, "pfc1", "pfc2", gp_pad)
            xmT_f32 = fc_stack(poolm_s, "mfc1", "mfc2", gm_pc)
            xmT = bpool.tile([128, gm_pc], f16, tag="xmT")
            nc.vector.tensor_copy(xmT[:], xmT_f32[:, :gm_pc])

            for half in range(gp_pad // 128):
                tp = psB.tile([128, 512], f32, tag="ps")
                nc.tensor.transpose(tp[:, :128],
                                    pT[:, half * 128:(half + 1) * 128], ident[:])
                rows = hpool.tile([128, 128], f32, tag="prow")
                nc.vector.tensor_copy(rows[:], tp[:, :128])
                nc.sync.dma_start(p_ag_in[half * 128:(half + 1) * 128, :],
                                  rows[:])
            ag(p_ag_in, p_full)

            # ---------------- PPI branch (replicated) ----------------
            pTf = bpool.tile([128, gp_tot], f16, tag="pTf")
            for t in range(ntok):
                rt = hpool.tile([128, 128], f32, tag="ppr")
                nc.sync.dma_start(rt[:], p_full[t * 128:(t + 1) * 128, :])
                tp = psB.tile([128, 512], f32, tag="ps")
                nc.tensor.transpose(tp[:, :128], rt[:], ident[:])
                nc.vector.tensor_copy(pTf[:, t * 128:(t + 1) * 128],
                                      tp[:, :128])

            def a_mult(h_tiles, wout, bn, relu, res_tiles):
                for s in range(gp_tot // 256):
                    ats = []
                    for ti in range(ntok):
                        at = atpool.tile([128, 256], f16, tag="at")
                        nc.sync.dma_start(
                            at[:], at_in[ti * 128:(ti + 1) * 128,
                                         s * 256:(s + 1) * 256])
                        ats.append(at)
                    for fch in range(wout // 128):
                        ps = psB.tile([128, 512], f32, tag="ps")
                        for ti in range(ntok):
                            nc.tensor.matmul(
                                ps[:, :256],
                                h_tiles[ti][:, fch * 128:(fch + 1) * 128],
                                ats[ti][:], start=(ti == 0),
                                stop=(ti == ntok - 1))
                        nc.scalar.activation(
                            res_tiles[fch][:, s * 256:(s + 1) * 256],
                            ps[:, :256], RELU if relu else IDENT,
                            bias=B[bn][:, fch:fch + 1])

            with tc.tile_pool(name="pq1", bufs=1) as pq1:
                q1T = [pq1.tile([128, gp_tot], f16, tag=f"q1T{i}",
                                name=f"q1T{i}") for i in range(8)]
                with tc.tile_pool(name="ph1", bufs=1) as ph1:
                    h1_tiles = []
                    for t in range(ntok):
                        ht = ph1.tile([128, 1024], f16, tag=f"h1r{t}")
                        for si in range(2):
                            ps = psB.tile([128, 512], f32, tag="ps")
                            nc.tensor.matmul(
                                ps[:], pTf[:, t * 128:(t + 1) * 128],
                                W["ppig1"][0][:, si * 512:(si + 1) * 512],
                                start=True, stop=True)
                            nc.vector.tensor_copy(
                                ht[:, si * 512:(si + 1) * 512], ps[:])
                        h1_tiles.append(ht)
                    a_mult(h1_tiles, 1024, "ppig1", True, q1T)

                h2_tiles = []
                for t in range(ntok):
                    ps = psB.tile([128, 512], f32, tag="ps")
                    for kch in range(8):
                        nc.tensor.matmul(
                            ps[:, :128], q1T[kch][:, t * 128:(t + 1) * 128],
                            W["ppig2"][kch][:],
                            start=(kch == 0), stop=(kch == 7))
                    ht = bpool.tile([128, 128], f16, tag=f"h2r{t}")
                    nc.vector.tensor_copy(ht[:], ps[:, :128])
                    h2_tiles.append(ht)
                q2T = bpool.tile([128, gp_tot], f16, tag="q2T")
                a_mult(h2_tiles, 128, "ppig2", True, [q2T])

            with tc.tile_pool(name="pfc1t", bufs=1) as pf:
                fc1T = [pf.tile([128, gp_tot], f16, tag=f"pfcT{i}",
                                name=f"pfcT{i}") for i in range(8)]
                for mch in range(8):
                    for s in range(nsl):
                        ps = psB.tile([128, 512], f32, tag="ps")
                        nc.tensor.matmul(
                            ps[:], W["ppifc1"][0][:, mch * 128:(mch + 1) * 128],
                            q2T[:, s * 512:(s + 1) * 512],
                            start=True, stop=True)
                        nc.scalar.activation(
                            fc1T[mch][:, s * 512:(s + 1) * 512], ps[:], RELU,
                            bias=B["ppifc1"][:, mch:mch + 1])
                for s in range(nsl):
                    ps = psB.tile([128, 512], f32, tag="ps")
                    for kch in range(8):
                        nc.tensor.matmul(ps[:], W["ppifc2"][kch][:],
                                         fc1T[kch][:, s * 512:(s + 1) * 512],
                                         start=(kch == 0), stop=(kch == 7))
                    qf = hpool.tile([128, 512], f32, tag="qfin")
                    nc.vector.tensor_scalar(
                        out=qf[:], in0=ps[:], scalar1=B["ppifc2"][:, 0:1],
                        scalar2=None, op0=mybir.AluOpType.add)
                    for j in range(4):
                        tp = psB.tile([128, 512], f32, tag="ps")
                        nc.tensor.transpose(tp[:, :128],
                                            qf[:, j * 128:(j + 1) * 128],
                                            ident[:])
                        rows = hpool.tile([128, 128], f32, tag="qrow")
                        nc.vector.tensor_copy(rows[:], tp[:, :128])
                        ti = s * 4 + j
                        nc.sync.dma_start(q_rows[ti * 128:(ti + 1) * 128, :],
                                          rows[:])

            q_selT = bpool.tile([128, gm_pc], f16, tag="qselT")
            for half in range(gm_pc // 128):
                qs = hpool.tile([128, 128], f32, tag="qsel")
                nc.gpsimd.indirect_dma_start(
                    out=qs[:], out_offset=None, in_=q_rows[:],
                    in_offset=bass.IndirectOffsetOnAxis(
                        ap=seq_t[:, half:half + 1], axis=0))
                tp = psB.tile([128, 512], f32, tag="ps")
                nc.tensor.transpose(tp[:, :128], qs[:], ident[:])
                nc.vector.tensor_copy(q_selT[:, half * 128:(half + 1) * 128],
                                      tp[:, :128])

            # ---------------- head ----------------
            hd1 = []
            for mch in range(8):
                ps = psB.tile([128, 512], f32, tag="ps")
                nc.tensor.matmul(ps[:, :gm_pc],
                                 W["fc1"][0][:, mch * 128:(mch + 1) * 128],
                                 xmT[:], start=True, stop=False)
                nc.tensor.matmul(ps[:, :gm_pc],
                                 W["fc1"][1][:, mch * 128:(mch + 1) * 128],
                                 q_selT[:], start=False, stop=True)
                t = bpool.tile([128, 512], f16, tag=f"hd1{mch}")
                nc.scalar.activation(t[:, :gm_pc], ps[:, :gm_pc], RELU,
                                     bias=B["fc1"][:, mch:mch + 1])
                hd1.append(t)
            hd2 = []
            for mch in range(4):
                ps = psB.tile([128, 512], f32, tag="ps")
                for kch in range(8):
                    nc.tensor.matmul(
                        ps[:, :gm_pc],
                        W["fc2"][kch][:, mch * 128:(mch + 1) * 128],
                        hd1[kch][:, :gm_pc], start=(kch == 0), stop=(kch == 7))
                t = bpool.tile([128, 512], f16, tag=f"hd2{mch}")
                nc.scalar.activation(t[:, :gm_pc], ps[:, :gm_pc], RELU,
                                     bias=B["fc2"][:, mch:mch + 1])
                hd2.append(t)
            ps = psB.tile([1, 512], f32, tag="ps")
            for kch in range(4):
                nc.tensor.matmul(ps[:, :gm_pc], W["out"][kch][:],
                                 hd2[kch][:, :gm_pc],
                                 start=(kch == 0), stop=(kch == 3))
            ot = hpool.tile([1, 512], f32, tag="outt")
            nc.vector.tensor_scalar(out=ot[:, :gm_pc], in0=ps[:, :gm_pc],
                                    scalar1=B["out"][:, 0:1], scalar2=None,
                                    op0=mybir.AluOpType.add)
            nc.sync.dma_start(out_par[:], ot[:, :gm_pc])
    nc.compile()
    return nc


# ----------------------------------------------------------------------------
# Entry
# ----------------------------------------------------------------------------

def _make_meta(inputs):
    seq_num = np.asarray(inputs['seq_num'])
    b_mol = seq_num.shape[0]
    b_pro = max(int(np.asarray(inputs['pro_batch']).max()) + 1,
                int(seq_num.max()) + 1,
                int(np.asarray(inputs['ppi_edge']).max()) + 1)
    b_pro = ((b_pro + NCORES - 1) // NCORES) * NCORES
    meta = dict(dims=DIMS)
    meta['pro'] = _prep_graph(np.asarray(inputs['pro_edge_index']),
                              np.asarray(inputs['pro_batch']), b_pro)
    meta['mol'] = _prep_graph(np.asarray(inputs['mol_edge_index']),
                              np.asarray(inputs['mol_batch']), b_mol)
    gpc = meta['pro']['gpc']
    meta['gp_pad'] = max(128, int(np.ceil(gpc / 128) * 128))
    meta['gm_pc'] = meta['mol']['gpc']
    meta['b_pro'] = b_pro
    meta['b_mol'] = b_mol
    meta['bias_shape'] = {n: list(_bias_host(n, inputs[BMAP[n]]).shape)
                          for n in DIMS}
    return meta


def _make_in_maps(inputs, meta):
    mp, mm = meta['pro'], meta['mol']
    gp_pad, gm_pc = meta['gp_pad'], meta['gm_pc']
    x_pro_pad = _pad_x(np.asarray(inputs['pro_x']), mp['pid'], mp['np_tot'])
    x_mol_pad = _pad_x(np.asarray(inputs['mol_x']), mm['pid'], mm['np_tot'])
    at, pg = _prep_ppi(np.asarray(inputs['ppi_edge']), meta['b_pro'],
                       mp['gpc'], gp_pad)
    seq = pg(np.asarray(inputs['seq_num']).astype(np.int64))
    iota = np.tile(np.arange(512, dtype=np.float16), (128, 1))
    weights = {("w_" + n): np.asarray(inputs[WMAP[n]]).astype(np.float16)
               for n in DIMS}
    biases = {("b_" + n): _bias_host(n, inputs[BMAP[n]]) for n in DIMS}

    in_maps = []
    for c in range(NCORES):
        m = {"x_pro": x_pro_pad, "x_mol": x_mol_pad, "iota": iota, "at": at}
        m["p_idx"], m["p_dst"], m["p_co"] = mp['src'][c], mp['dstl'][c], mp['coe'][c]
        m["p_bl"], m["p_ic"] = mp['bl'][c], mp['ic'][c]
        m["m_idx"], m["m_dst"], m["m_co"] = mm['src'][c], mm['dstl'][c], mm['coe'][c]
        m["m_bl"], m["m_ic"] = mm['bl'][c], mm['ic'][c]
        sq = seq[c * gm_pc:(c + 1) * gm_pc].astype(np.int32)
        m["seq"] = np.ascontiguousarray(sq.reshape(-1, 128).T)
        m.update(weights)
        m.update(biases)
        in_maps.append(m)
    return in_maps


def kernel(**inputs):
    sig = (np.asarray(inputs['mol_x']).shape,
           np.asarray(inputs['pro_x']).shape,
           np.asarray(inputs['mol_edge_index'])[:, :64].tobytes(),
           np.asarray(inputs['pro_edge_index'])[:, :64].tobytes(),
           np.asarray(inputs['seq_num'])[:16].tobytes())
    if sig in _CACHE:
        runner, meta = _CACHE[sig]
    else:
        meta = _make_meta(inputs)
        nc = _build_program(meta)
        runner = SpmdRunner(nc, NCORES)
        _CACHE[sig] = (runner, meta)
    in_maps = _make_in_maps(inputs, meta)
    runner.put_inputs(in_maps)
    results = runner.results(runner.run())
    return np.concatenate(
        [results[c]["out"][0] for c in range(NCORES)]).astype(np.float32)[:, None]


# revision 13
# speedup vs baseline: 1.3756x; 1.3756x over previous
"""Trainium2 Bass kernel for nn_BUNet (GCN mol+pro branches, PPI branch, head).

Self-contained: host graph preprocessing + SPMD Bass/Tile program on 8
NeuronCores + output assembly.  Sharding: graph-aligned node shards per core,
edges partitioned by destination; per-layer fp16 AllGather of hidden tables;
PPI branch replicated with dense normalized adjacency; head sharded by pair.
"""
import sys
sys.path.insert(0, '/opt/trn_rl_repo')
import numpy as np

from concourse import bass, mybir
import concourse.bacc as bacc
import concourse.tile as tile
from concourse.masks import make_identity
from concourse.tile import add_dep_helper

NCORES = 8
SLAB = 256        # chunks of edge-metadata per streaming DMA
f16 = mybir.dt.float16
f32 = mybir.dt.float32
i32 = mybir.dt.int32
RELU = mybir.ActivationFunctionType.Relu
IDENT = mybir.ActivationFunctionType.Identity

_CACHE = {}


# ----------------------------------------------------------------------------
# Embedded SPMD runner (PJRT path, persistent jit)
# ----------------------------------------------------------------------------

class SpmdRunner:
    def __init__(self, nc, n_cores):
        import jax
        from jax.sharding import Mesh, PartitionSpec
        from jax.experimental.shard_map import shard_map
        from concourse.bass2jax import (_bass_exec_p, install_neuronx_cc_hook,
                                        partition_id_tensor)
        self.jax = jax
        install_neuronx_cc_hook()
        self.nc = nc
        self.n_cores = n_cores
        partition_name = (nc.partition_id_tensor.name
                          if nc.partition_id_tensor else None)
        in_names, out_names, out_avals, zero_outs = [], [], [], []
        for alloc in nc.m.functions[0].allocations:
            if not isinstance(alloc, mybir.MemoryLocationSet):
                continue
            name = alloc.memorylocations[0].name
            if alloc.kind == "ExternalInput":
                if name != partition_name:
                    in_names.append(name)
            elif alloc.kind == "ExternalOutput":
                out_names.append(name)
                shape = tuple(alloc.tensor_shape)
                dtype = mybir.dt.np(alloc.dtype)
                out_avals.append(jax.core.ShapedArray(shape, dtype))
                zero_outs.append(np.zeros(shape, dtype))
        self.in_names = list(in_names)
        self.out_names = out_names
        self.out_avals = out_avals
        self.zero_outs = zero_outs
        n_params = len(self.in_names)
        n_outs = len(out_names)
        all_in_names = self.in_names + out_names
        if partition_name is not None:
            all_in_names.append(partition_name)

        def _body(*args):
            operands = list(args)
            if partition_name is not None:
                operands.append(partition_id_tensor())
            outs = _bass_exec_p.bind(
                *operands, out_avals=tuple(out_avals),
                in_names=tuple(all_in_names), out_names=tuple(out_names),
                lowering_input_output_aliases=(), sim_require_finite=True,
                sim_require_nnan=True, nc=nc)
            return tuple(outs)

        devices = jax.devices()[:n_cores]
        self.mesh = Mesh(np.asarray(devices), ("core",))
        in_specs = (PartitionSpec("core"),) * (n_params + n_outs)
        out_specs = (PartitionSpec("core"),) * n_outs
        donate = tuple(range(n_params, n_params + n_outs))
        self.fn = jax.jit(
            shard_map(_body, mesh=self.mesh, in_specs=in_specs,
                      out_specs=out_specs, check_rep=False),
            donate_argnums=donate, keep_unused=True)
        self.resident = None

    def put_inputs(self, in_maps):
        from jax.sharding import NamedSharding, PartitionSpec
        concat = [
            np.concatenate([np.asarray(in_maps[c][n])
                            for c in range(self.n_cores)], axis=0)
            for n in self.in_names]
        sh = NamedSharding(self.mesh, PartitionSpec("core"))
        self.resident = [self.jax.device_put(a, sh) for a in concat]

    def run(self):
        zeros = [np.zeros((self.n_cores * z.shape[0], *z.shape[1:]), z.dtype)
                 for z in self.zero_outs]
        out = self.fn(*self.resident, *zeros)
        self.jax.block_until_ready(out)
        return out

    def results(self, outs):
        res = []
        for c in range(self.n_cores):
            d = {}
            for i, name in enumerate(self.out_names):
                d[name] = np.asarray(outs[i]).reshape(
                    self.n_cores, *self.out_avals[i].shape)[c]
            res.append(d)
        return res


# ----------------------------------------------------------------------------
# Host preprocessing
# ----------------------------------------------------------------------------

def _prep_graph(edge_index, batch, n_graphs):
    """Graph-aligned node sharding + dst-partitioned chunked edges."""
    n = batch.shape[0]
    src = edge_index[0].astype(np.int64)
    dst = edge_index[1].astype(np.int64)
    loops = np.arange(n, dtype=np.int64)
    src_a = np.concatenate([src, loops])
    dst_a = np.concatenate([dst, loops])
    deg = np.bincount(dst_a, minlength=n).astype(np.float64)
    dis = 1.0 / np.sqrt(np.maximum(deg, 1.0))
    dis[deg <= 0] = 0.0
    coeff = (dis[src_a] * dis[dst_a]).astype(np.float32)

    gpc = n_graphs // NCORES
    node_core = (batch.astype(np.int64) // gpc).clip(0, NCORES - 1)
    shard_start = np.searchsorted(node_core, np.arange(NCORES))
    shard_end = np.searchsorted(node_core, np.arange(NCORES), side='right')
    shard_size = shard_end - shard_start
    s_max = int(np.ceil(max(shard_size.max(), 1) / 128) * 128)
    ntiles = s_max // 128
    np_tot = NCORES * s_max

    pid = np.empty(n, dtype=np.int64)
    for r in range(NCORES):
        sl = slice(shard_start[r], shard_end[r])
        pid[sl] = r * s_max + np.arange(shard_size[r])

    src_p = pid[src_a]
    dst_p = pid[dst_a]
    e_core = dst_p // s_max
    dst_tile_local = (dst_p % s_max) // 128

    cnt = np.zeros((NCORES, ntiles), dtype=np.int64)
    order_all = []
    for r in range(NCORES):
        m = np.nonzero(e_core == r)[0]
        o = m[np.argsort(dst_p[m], kind='stable')]
        order_all.append(o)
        c = np.bincount(dst_tile_local[o], minlength=ntiles)
        cnt[r] = np.ceil(c / 128).astype(np.int64)
    cnt_g = np.maximum(cnt.max(axis=0), 1)
    cnt_g[-1] += (-int(cnt_g.sum())) % 16
    c_tot = int(cnt_g.sum())
    tile_ofs = np.concatenate([[0], np.cumsum(cnt_g)])

    srcs, dstls, coes = [], [], []
    for r in range(NCORES):
        o = order_all[r]
        t = dst_tile_local[o]
        sarr = np.zeros((c_tot * 128,), dtype=np.int32)
        darr = np.zeros((c_tot * 128,), dtype=np.float32)
        carr = np.zeros((c_tot * 128,), dtype=np.float32)
        tile_cnt = np.bincount(t, minlength=ntiles)
        pos = 0
        for ti in range(ntiles):
            k = int(tile_cnt[ti])
            sel = o[pos:pos + k]
            pos += k
            base = int(tile_ofs[ti]) * 128
            sarr[base:base + k] = src_p[sel].astype(np.int32)
            darr[base:base + k] = (dst_p[sel] % 128).astype(np.float32)
            carr[base:base + k] = coeff[sel]
        srcs.append(np.ascontiguousarray(sarr.reshape(c_tot, 128).T))
        dstls.append(np.ascontiguousarray(darr.reshape(c_tot, 128).T))
        coes.append(np.ascontiguousarray(carr.reshape(c_tot, 128).T))

    gcnt = np.bincount(batch.astype(np.int64), minlength=n_graphs).astype(np.float64)
    inv = np.where(gcnt > 0, 1.0 / np.maximum(gcnt, 1.0), 0.0)
    bls, ics = [], []
    for r in range(NCORES):
        bl = np.zeros((s_max,), dtype=np.float32)
        ic = np.zeros((s_max,), dtype=np.float32)
        sl = slice(shard_start[r], shard_end[r])
        sz = int(shard_size[r])
        bidx = batch[sl].astype(np.int64)
        bl[:sz] = (bidx - r * gpc).astype(np.float32)
        ic[:sz] = inv[bidx].astype(np.float32)
        bls.append(np.ascontiguousarray(bl.reshape(ntiles, 128).T))
        ics.append(np.ascontiguousarray(ic.reshape(ntiles, 128).T))

    return dict(gpc=gpc, s_max=s_max, ntiles=ntiles, np_tot=np_tot,
                cnt_g=cnt_g, c_tot=c_tot, pid=pid,
                src=srcs, dstl=dstls, coe=coes, bl=bls, ic=ics)


def _pad_x(x, pid, np_tot):
    out = np.zeros((np_tot, x.shape[1]), dtype=np.float16)
    out[pid] = x.astype(np.float16)
    return out


def _prep_ppi(ppi_edge, b_pro, gpc, gp_pad):
    gp_tot = NCORES * gp_pad
    qs = ppi_edge[0].astype(np.int64)
    qd = ppi_edge[1].astype(np.int64)
    deg = np.bincount(qd, minlength=b_pro) + 1.0
    dis = 1.0 / np.sqrt(deg)

    def pg(g):
        return (g // gpc) * gp_pad + (g % gpc)

    A = np.zeros((gp_tot, gp_tot), dtype=np.float32)
    np.add.at(A, (pg(qd), pg(qs)), (dis[qd] * dis[qs]).astype(np.float32))
    gids = np.arange(b_pro, dtype=np.int64)
    A[pg(gids), pg(gids)] += (dis * dis).astype(np.float32)
    return np.ascontiguousarray(A.T).astype(np.float16), pg


# ----------------------------------------------------------------------------
# Device program
# ----------------------------------------------------------------------------

DIMS = dict(pg1=(33, 128), pg2=(128, 128), pg3=(128, 128),
            mg1=(78, 156), mg2=(156, 312), mg3=(312, 128),
            pfc1=(128, 1024), pfc2=(1024, 128),
            mfc1=(128, 1024), mfc2=(1024, 128),
            ppig1=(128, 1024), ppig2=(1024, 128),
            ppifc1=(128, 1024), ppifc2=(1024, 128),
            fc1=(256, 1024), fc2=(1024, 512), out=(512, 1))

WMAP = dict(pg1="w_pg1", pg2="w_pg2", pg3="w_pg3", mg1="w_mg1", mg2="w_mg2",
            mg3="w_mg3", pfc1="w_pfc1", pfc2="w_pfc2", mfc1="w_mfc1",
            mfc2="w_mfc2", ppig1="w_ppig1", ppig2="w_ppig2",
            ppifc1="w_ppifc1", ppifc2="w_ppifc2", fc1="w_fc1", fc2="w_fc2",
            out="w_out")
BMAP = {k: "b" + v[1:] for k, v in WMAP.items()}
REPL_BIAS = {"pg1", "pg2", "pg3", "mg1", "mg2", "mg3"}


def _bias_host(name, b):
    b = np.asarray(b)
    if name in REPL_BIAS:
        return np.tile(b.astype(np.float32)[None, :], (128, 1))
    n = b.shape[0]
    if n % 128 == 0:
        return np.ascontiguousarray(b.astype(np.float32).reshape(-1, 128).T)
    assert n == 1
    return b.astype(np.float32).reshape(1, 1)


def _kg_for(ch):
    # HW indirect DMA consumes one index per partition (contiguous free dim),
    # so multi-chunk gathers are incorrect on hardware; keep one chunk per DMA.
    return 1


def _build_program(meta):
    mp, mm = meta['pro'], meta['mol']
    gp_pad = meta['gp_pad']
    gm_pc = meta['gm_pc']
    gp_tot = NCORES * gp_pad
    ntok = gp_tot // 128
    nsl = gp_tot // 512

    nc = bacc.Bacc(None, target_bir_lowering=False, debug=False)

    def par(name, shape, dt):
        return nc.declare_dram_parameter(name, list(shape), dt, isOutput=False)

    x_pro = par("x_pro", (mp['np_tot'], 33), f16)
    x_mol = par("x_mol", (mm['np_tot'], 78), f16)
    e_in = {}
    for b, m in (("p", mp), ("m", mm)):
        e_in[b + "idx"] = par(b + "_idx", (128, m['c_tot']), i32)
        e_in[b + "dst"] = par(b + "_dst", (128, m['c_tot']), f32)
        e_in[b + "co"] = par(b + "_co", (128, m['c_tot']), f32)
        e_in[b + "bl"] = par(b + "_bl", (128, m['ntiles']), f32)
        e_in[b + "ic"] = par(b + "_ic", (128, m['ntiles']), f32)
    iota_in = par("iota", (128, 512), f16)
    at_in = par("at", (gp_tot, gp_tot), f16)
    seq_in = par("seq", (128, gm_pc // 128), i32)
    w_in, wb_in = {}, {}
    for n, (a, b) in DIMS.items():
        w_in[n] = par("w_" + n, (a, b), f16)
        wb_in[n] = par("b_" + n, meta['bias_shape'][n], f32)
    out_par = nc.declare_dram_parameter("out", [1, gm_pc], f32, isOutput=True)

    h_p1_in = nc.dram_tensor("hp1i", [mp['s_max'], 128], f16)
    h_p2_in = nc.dram_tensor("hp2i", [mp['s_max'], 128], f16)
    h_m1_in = nc.dram_tensor("hm1i", [mm['s_max'], 156], f16)
    h_m2_in = nc.dram_tensor("hm2i", [mm['s_max'], 312], f16)
    h_p1 = nc.dram_tensor("hp1", [mp['np_tot'], 128], f16, addr_space="Shared")
    h_p2 = nc.dram_tensor("hp2", [mp['np_tot'], 128], f16, addr_space="Shared")
    h_m1 = nc.dram_tensor("hm1", [mm['np_tot'], 156], f16, addr_space="Shared")
    h_m2 = nc.dram_tensor("hm2", [mm['np_tot'], 312], f16, addr_space="Shared")
    p_ag_in = nc.dram_tensor("pagi", [gp_pad, 128], f32)
    p_full = nc.dram_tensor("pfull", [gp_tot, 128], f32, addr_space="Shared")
    q_rows = nc.dram_tensor("qrows", [gp_tot, 128], f32)
    RG = [list(range(NCORES))]

    with tile.TileContext(nc, num_cores=NCORES) as tc:
        with (
            tc.tile_pool(name="const", bufs=1) as cpool,
            tc.tile_pool(name="meta", bufs=2) as ipool,
            tc.tile_pool(name="g", bufs=6) as gpool,
            tc.tile_pool(name="s", bufs=6) as spool,
            tc.tile_pool(name="aggs", bufs=3) as apool,
            tc.tile_pool(name="h", bufs=3) as hpool,
            tc.tile_pool(name="big", bufs=1) as bpool,
            tc.tile_pool(name="at", bufs=18) as atpool,
            tc.tile_pool(name="psA", bufs=3, space="PSUM") as psA,
            tc.tile_pool(name="psB", bufs=3, space="PSUM") as psB,
            tc.tile_pool(name="psPool", bufs=1, space="PSUM") as psP,
        ):
            # ---------------- constants ----------------
            iota = cpool.tile([128, 512], f16)
            nc.sync.dma_start(iota[:], iota_in[:])
            ident = cpool.tile([128, 128], f32)
            make_identity(nc, ident[:])
            W, B = {}, {}
            for n, (a, b) in DIMS.items():
                tiles = []
                for j in range((a + 127) // 128):
                    aj = min(128, a - j * 128)
                    t = cpool.tile([aj, b], f16, tag=f"w{n}{j}")
                    nc.sync.dma_start(t[:], w_in[n][j * 128:j * 128 + aj, :])
                    tiles.append(t)
                W[n] = tiles
                shp = meta['bias_shape'][n]
                bt = cpool.tile(list(shp), f32, tag=f"b{n}")
                nc.sync.dma_start(bt[:], wb_in[n][:])
                B[n] = bt
            bl_p = cpool.tile([128, mp['ntiles']], f32)
            ic_p = cpool.tile([128, mp['ntiles']], f32)
            bl_m = cpool.tile([128, mm['ntiles']], f32)
            ic_m = cpool.tile([128, mm['ntiles']], f32)
            nc.sync.dma_start(bl_p[:], e_in["pbl"][:])
            nc.sync.dma_start(ic_p[:], e_in["pic"][:])
            nc.sync.dma_start(bl_m[:], e_in["mbl"][:])
            nc.sync.dma_start(ic_m[:], e_in["mic"][:])
            seq_t = cpool.tile([128, gm_pc // 128], i32)
            nc.sync.dma_start(seq_t[:], seq_in[:])

            pool_p = psP.tile([128, gp_pad], f32, tag="poolP")
            pool_m = psP.tile([128, gm_pc], f32, tag="poolM")

            # ---------------- GCN layer ----------------
            def gcn(branch, lname, ch, feat, table, wn, ag_in=None,
                    pool_target=None, pool_bl=None, pool_ic=None, pool_w=0):
                m = mp if branch == "p" else mm
                cnt_g, ntiles, c_tot = m['cnt_g'], m['ntiles'], m['c_tot']
                idx_in, dst_in, co_in = (e_in[branch + "idx"],
                                         e_in[branch + "dst"],
                                         e_in[branch + "co"])
                kg = _kg_for(ch)
                nch = (ch + 127) // 128
                aggw = 128 * nch
                tile_of_chunk = np.repeat(np.arange(ntiles), cnt_g)
                tile_ofs = np.concatenate([[0], np.cumsum(cnt_g)])
                it = dt_ = ct = None
                gt, gbase = None, 0
                agg_p = None
                for c in range(c_tot):
                    if c % SLAB == 0:
                        w = min(SLAB, c_tot - c)
                        it = ipool.tile([128, SLAB], i32, tag="eidx")
                        dt_ = ipool.tile([128, SLAB], f32, tag="edst")
                        ct = ipool.tile([128, SLAB], f32, tag="eco")
                        nc.sync.dma_start(it[:, :w], idx_in[:, c:c + w])
                        nc.sync.dma_start(dt_[:, :w], dst_in[:, c:c + w])
                        nc.sync.dma_start(ct[:, :w], co_in[:, c:c + w])
                    if c % kg == 0:
                        gt = gpool.tile([128, 384], f16, tag="g")
                        so = c % SLAB
                        nc.gpsimd.indirect_dma_start(
                            out=gt[:, :kg * ch], out_offset=None, in_=table[:],
                            in_offset=bass.IndirectOffsetOnAxis(
                                ap=it[:, so:so + kg], axis=0))
                        tail = (128 - ch % 128) % 128
                        if nch > 1 and tail:
                            nc.vector.memset(
                                gt[:, kg * ch:kg * ch + tail], 0.0)
                        gbase = c
                    t = int(tile_of_chunk[c])
                    first = (c == tile_ofs[t])
                    last = (c == tile_ofs[t + 1] - 1)
                    if first:
                        agg_p = psA.tile([128, 384], f32, tag="agg")
                    st = spool.tile([128, 256], f16, tag="s")
                    so = c % SLAB
                    nc.vector.tensor_scalar(
                        out=st[:, :128], in0=iota[:, :128],
                        scalar1=dt_[:, so:so + 1], scalar2=ct[:, so:so + 1],
                        op0=mybir.AluOpType.is_equal, op1=mybir.AluOpType.mult)
                    co = (c - gbase) * ch
                    if nch == 1:
                        nc.tensor.matmul(
                            agg_p[0:ch, 0:128], gt[:, co:co + ch],
                            st[:, :128], start=first, stop=last)
                    else:
                        for j in range(nch):
                            nc.tensor.matmul(
                                agg_p[:, j * 128:j * 128 + 128],
                                gt[:, co + j * 128: co + j * 128 + 128],
                                st[:, :128], start=(first and j == 0),
                                stop=(last and j == nch - 1))
                    if last:
                        agg_s = apool.tile([128, 384], f16, tag="aggs")
                        if nch == 1:
                            nc.vector.tensor_copy(agg_s[0:ch, :128],
                                                  agg_p[0:ch, :128])
                        else:
                            nc.vector.tensor_copy(agg_s[:, :aggw],
                                                  agg_p[:, :aggw])
                        out_p = psB.tile([128, 512], f32, tag="ps")
                        for j in range(nch):
                            chj = min(128, ch - j * 128)
                            nc.tensor.matmul(
                                out_p[:, :feat],
                                agg_s[0:chj, j * 128:j * 128 + 128],
                                W[wn][j][:], start=(j == 0), stop=(j == nch - 1))
                        h_t = hpool.tile([128, 384], f16, tag="gh")
                        nc.vector.tensor_tensor(
                            out=h_t[:, :feat], in0=out_p[:, :feat],
                            in1=B[wn][:, :feat], op=mybir.AluOpType.add)
                        nc.scalar.activation(h_t[:, :feat], h_t[:, :feat], RELU)
                        if ag_in is not None:
                            nc.sync.dma_start(
                                ag_in[t * 128:(t + 1) * 128, :], h_t[:, :feat])
                        if pool_target is not None:
                            sp = spool.tile([128, 256], f16, tag="s")
                            nc.vector.tensor_scalar(
                                out=sp[:, :pool_w], in0=iota[:, :pool_w],
                                scalar1=pool_bl[:, t:t + 1],
                                scalar2=pool_ic[:, t:t + 1],
                                op0=mybir.AluOpType.is_equal,
                                op1=mybir.AluOpType.mult)
                            nc.tensor.matmul(
                                pool_target[:], h_t[:, :feat], sp[:, :pool_w],
                                start=(t == 0), stop=(t == ntiles - 1))

            def ag(src_t, dst_t):
                nc.gpsimd.collective_compute(
                    "AllGather", mybir.AluOpType.bypass, replica_groups=RG,
                    ins=[src_t[:]], outs=[dst_t[:]])

            # ---------------- schedule ----------------
            gcn("p", "pL1", 33, 128, x_pro, "pg1", ag_in=h_p1_in)
            ag(h_p1_in, h_p1)
            gcn("m", "mL1", 78, 156, x_mol, "mg1", ag_in=h_m1_in)
            ag(h_m1_in, h_m1)
            gcn("p", "pL2", 128, 128, h_p1, "pg2", ag_in=h_p2_in)
            ag(h_p2_in, h_p2)
            gcn("m", "mL2", 156, 312, h_m1, "mg2", ag_in=h_m2_in)
            ag(h_m2_in, h_m2)
            gcn("p", "pL3", 128, 128, h_p2, "pg3", pool_target=pool_p,
                pool_bl=bl_p, pool_ic=ic_p, pool_w=gp_pad)
            gcn("m", "mL3", 312, 128, h_m2, "mg3", pool_target=pool_m,
                pool_bl=bl_m, pool_ic=ic_m, pool_w=gm_pc)

            # ---------------- FC stacks (feature-major) ----------------
            def fc_stack(poolt, w1n, w2n, width):
                p1 = []
                for mch in range(8):
                    ps = psB.tile([128, 512], f32, tag="ps")
                    nc.tensor.matmul(ps[:, :width],
                                     W[w1n][0][:, mch * 128:(mch + 1) * 128],
                                     poolt[:], start=True, stop=True)
                    t = bpool.tile([128, 512], f16, tag=f"fcs{mch}")
                    nc.scalar.activation(t[:, :width], ps[:, :width], RELU,
                                         bias=B[w1n][:, mch:mch + 1])
                    p1.append(t)
                ps = psB.tile([128, 512], f32, tag="ps")
                for kch in range(8):
                    nc.tensor.matmul(ps[:, :width], W[w2n][kch][:],
                                     p1[kch][:, :width],
                                     start=(kch == 0), stop=(kch == 7))
                t = hpool.tile([128, 512], f32, tag="fco")
                nc.vector.tensor_scalar(out=t[:, :width], in0=ps[:, :width],
                                        scalar1=B[w2n][:, 0:1], scalar2=None,
                                        op0=mybir.AluOpType.add)
                return t

            poolp_s = bpool.tile([128, gp_pad], f16, tag="poolps")
            nc.vector.tensor_copy(poolp_s[:], pool_p[:])
            poolm_s = bpool.tile([128, gm_pc], f16, tag="poolms")
            nc.vector.tensor_copy(poolm_s[:], pool_m[:])
            pT = fc_stack(poolp_s, "pfc1", "pfc2", gp_pad)
            xmT_f32 = fc_stack(poolm_s, "mfc1", "mfc2", gm_pc)
            xmT = bpool.tile([128, gm_pc], f16, tag="xmT")
            nc.vector.tensor_copy(xmT[:], xmT_f32[:, :gm_pc])

            for half in range(gp_pad // 128):
                tp = psB.tile([128, 512], f32, tag="ps")
                nc.tensor.transpose(tp[:, :128],
                                    pT[:, half * 128:(half + 1) * 128], ident[:])
                rows = hpool.tile([128, 128], f32, tag="prow")
                nc.vector.tensor_copy(rows[:], tp[:, :128])
                nc.sync.dma_start(p_ag_in[half * 128:(half + 1) * 128, :],
                                  rows[:])
            ag(p_ag_in, p_full)

            # ---------------- PPI branch (replicated) ----------------
            pTf = bpool.tile([128, gp_tot], f16, tag="pTf")
            for t in range(ntok):
                rt = hpool.tile([128, 128], f32, tag="ppr")
                nc.sync.dma_start(rt[:], p_full[t * 128:(t + 1) * 128, :])
                tp = psB.tile([128, 512], f32, tag="ps")
                nc.tensor.transpose(tp[:, :128], rt[:], ident[:])
                nc.vector.tensor_copy(pTf[:, t * 128:(t + 1) * 128],
                                      tp[:, :128])

            def a_mult(h_tiles, wout, bn, relu, res_tiles):
                for s in range(gp_tot // 256):
                    ats = []
                    for ti in range(ntok):
                        at = atpool.tile([128, 256], f16, tag="at")
                        nc.sync.dma_start(
                            at[:], at_in[ti * 128:(ti + 1) * 128,
                                         s * 256:(s + 1) * 256])
                        ats.append(at)
                    for fch in range(wout // 128):
                        ps = psB.tile([128, 512], f32, tag="ps")
                        for ti in range(ntok):
                            nc.tensor.matmul(
                                ps[:, :256],
                                h_tiles[ti][:, fch * 128:(fch + 1) * 128],
                                ats[ti][:], start=(ti == 0),
                                stop=(ti == ntok - 1))
                        nc.scalar.activation(
                            res_tiles[fch][:, s * 256:(s + 1) * 256],
                            ps[:, :256], RELU if relu else IDENT,
                            bias=B[bn][:, fch:fch + 1])

            with tc.tile_pool(name="pq1", bufs=1) as pq1:
                q1T = [pq1.tile([128, gp_tot], f16, tag=f"q1T{i}",
                                name=f"q1T{i}") for i in range(8)]
                with tc.tile_pool(name="ph1", bufs=1) as ph1:
                    h1_tiles = []
                    for t in range(ntok):
                        ht = ph1.tile([128, 1024], f16, tag=f"h1r{t}")
                        for si in range(2):
                            ps = psB.tile([128, 512], f32, tag="ps")
                            nc.tensor.matmul(
                                ps[:], pTf[:, t * 128:(t + 1) * 128],
                                W["ppig1"][0][:, si * 512:(si + 1) * 512],
                                start=True, stop=True)
                            nc.vector.tensor_copy(
                                ht[:, si * 512:(si + 1) * 512], ps[:])
                        h1_tiles.append(ht)
                    a_mult(h1_tiles, 1024, "ppig1", True, q1T)

                h2_tiles = []
                for t in range(ntok):
                    ps = psB.tile([128, 512], f32, tag="ps")
                    for kch in range(8):
                        nc.tensor.matmul(
                            ps[:, :128], q1T[kch][:, t * 128:(t + 1) * 128],
                            W["ppig2"][kch][:],
                            start=(kch == 0), stop=(kch == 7))
                    ht = bpool.tile([128, 128], f16, tag=f"h2r{t}")
                    nc.vector.tensor_copy(ht[:], ps[:, :128])
                    h2_tiles.append(ht)
                q2T = bpool.tile([128, gp_tot], f16, tag="q2T")
                a_mult(h2_tiles, 128, "ppig2", True, [q2T])

            with tc.tile_pool(name="pfc1t", bufs=1) as pf:
                fc1T = [pf.tile([128, gp_tot], f16, tag=f"pfcT{i}",
                                name=f"pfcT{i}") for i in range(8)]
                for mch in range(8):
                    for s in range(nsl):
                        ps = psB.tile([128, 512], f32, tag="ps")
                        nc.tensor.matmul(
                            ps[:], W["ppifc1"][0][:, mch * 128:(mch + 1) * 128],
                            q2T[:, s * 512:(s + 1) * 512],
                            start=True, stop=True)
                        nc.scalar.activation(
                            fc1T[mch][:, s * 512:(s + 1) * 512], ps[:], RELU,
                            bias=B["ppifc1"][:, mch:mch + 1])
                for s in range(nsl):
                    ps = psB.tile([128, 512], f32, tag="ps")
                    for kch in range(8):
                        nc.tensor.matmul(ps[:], W["ppifc2"][kch][:],
                                         fc1T[kch][:, s * 512:(s + 1) * 512],
                                         start=(kch == 0), stop=(kch == 7))
                    qf = hpool.tile([128, 512], f32, tag="qfin")
                    nc.vector.tensor_scalar(
                        out=qf[:], in0=ps[:], scalar1=B["ppifc2"][:, 0:1],
                        scalar2=None, op0=mybir.AluOpType.add)
                    for j in range(4):
                        tp = psB.tile([128, 512], f32, tag="ps")
                        nc.tensor.transpose(tp[:, :128],
                                            qf[:, j * 128:(j + 1) * 128],
                                            ident[:])
                        rows = hpool.tile([128, 128], f32, tag="qrow")
                        nc.vector.tensor_copy(rows[:], tp[:, :128])
                        ti = s * 4 + j
                        nc.sync.dma_start(q_rows[ti * 128:(ti + 1) * 128, :],
                                          rows[:])

            q_selT = bpool.tile([128, gm_pc], f16, tag="qselT")
            for half in range(gm_pc // 128):
                qs = hpool.tile([128, 128], f32, tag="qsel")
                nc.gpsimd.indirect_dma_start(
                    out=qs[:], out_offset=None, in_=q_rows[:],
                    in_offset=bass.IndirectOffsetOnAxis(
                        ap=seq_t[:, half:half + 1], axis=0))
                tp = psB.tile([128, 512], f32, tag="ps")
                nc.tensor.transpose(tp[:, :128], qs[:], ident[:])
                nc.vector.tensor_copy(q_selT[:, half * 128:(half + 1) * 128],
                                      tp[:, :128])

            # ---------------- head ----------------
            hd1 = []
            for mch in range(8):
                ps = psB.tile([128, 512], f32, tag="ps")
                nc.tensor.matmul(ps[:, :gm_pc],
                                 W["fc1"][0][:, mch * 128:(mch + 1) * 128],
                                 xmT[:], start=True, stop=False)
                nc.tensor.matmul(ps[:, :gm_pc],
                                 W["fc1"][1][:, mch * 128:(mch + 1) * 128],
                                 q_selT[:], start=False, stop=True)
                t = bpool.tile([128, 512], f16, tag=f"hd1{mch}")
                nc.scalar.activation(t[:, :gm_pc], ps[:, :gm_pc], RELU,
                                     bias=B["fc1"][:, mch:mch + 1])
                hd1.append(t)
            hd2 = []
            for mch in range(4):
                ps = psB.tile([128, 512], f32, tag="ps")
                for kch in range(8):
                    nc.tensor.matmul(
                        ps[:, :gm_pc],
                        W["fc2"][kch][:, mch * 128:(mch + 1) * 128],
                        hd1[kch][:, :gm_pc], start=(kch == 0), stop=(kch == 7))
                t = bpool.tile([128, 512], f16, tag=f"hd2{mch}")
                nc.scalar.activation(t[:, :gm_pc], ps[:, :gm_pc], RELU,
                                     bias=B["fc2"][:, mch:mch + 1])
                hd2.append(t)
            ps = psB.tile([1, 512], f32, tag="ps")
            for kch in range(4):
                nc.tensor.matmul(ps[:, :gm_pc], W["out"][kch][:],
                                 hd2[kch][:, :gm_pc],
                                 start=(kch == 0), stop=(kch == 3))
            ot = hpool.tile([1, 512], f32, tag="outt")
            nc.vector.tensor_scalar(out=ot[:, :gm_pc], in0=ps[:, :gm_pc],
                                    scalar1=B["out"][:, 0:1], scalar2=None,
                                    op0=mybir.AluOpType.add)
            nc.sync.dma_start(out_par[:], ot[:, :gm_pc])
    nc.compile()
    return nc


# ----------------------------------------------------------------------------
# Entry
# ----------------------------------------------------------------------------

def _make_meta(inputs):
    seq_num = np.asarray(inputs['seq_num'])
    b_mol = seq_num.shape[0]
    b_pro = max(int(np.asarray(inputs['pro_batch']).max()) + 1,
                int(seq_num.max()) + 1,
                int(np.asarray(inputs['ppi_edge']).max()) + 1)
    b_pro = ((b_pro + NCORES - 1) // NCORES) * NCORES
    meta = dict(dims=DIMS)
    meta['pro'] = _prep_graph(np.asarray(inputs['pro_edge_index']),
                              np.asarray(inputs['pro_batch']), b_pro)
    meta['mol'] = _prep_graph(np.asarray(inputs['mol_edge_index']),
                              np.asarray(inputs['mol_batch']), b_mol)
    gpc = meta['pro']['gpc']
    meta['gp_pad'] = max(128, int(np.ceil(gpc / 128) * 128))
    meta['gm_pc'] = meta['mol']['gpc']
    meta['b_pro'] = b_pro
    meta['b_mol'] = b_mol
    meta['bias_shape'] = {n: list(_bias_host(n, inputs[BMAP[n]]).shape)
                          for n in DIMS}
    return meta


def _make_in_maps(inputs, meta):
    mp, mm = meta['pro'], meta['mol']
    gp_pad, gm_pc = meta['gp_pad'], meta['gm_pc']
    x_pro_pad = _pad_x(np.asarray(inputs['pro_x']), mp['pid'], mp['np_tot'])
    x_mol_pad = _pad_x(np.asarray(inputs['mol_x']), mm['pid'], mm['np_tot'])
    at, pg = _prep_ppi(np.asarray(inputs['ppi_edge']), meta['b_pro'],
                       mp['gpc'], gp_pad)
    seq = pg(np.asarray(inputs['seq_num']).astype(np.int64))
    iota = np.tile(np.arange(512, dtype=np.float16), (128, 1))
    weights = {("w_" + n): np.asarray(inputs[WMAP[n]]).astype(np.float16)
               for n in DIMS}
    biases = {("b_" + n): _bias_host(n, inputs[BMAP[n]]) for n in DIMS}

    in_maps = []
    for c in range(NCORES):
        m = {"x_pro": x_pro_pad, "x_mol": x_mol_pad, "iota": iota, "at": at}
        m["p_idx"], m["p_dst"], m["p_co"] = mp['src'][c], mp['dstl'][c], mp['coe'][c]
        m["p_bl"], m["p_ic"] = mp['bl'][c], mp['ic'][c]
        m["m_idx"], m["m_dst"], m["m_co"] = mm['src'][c], mm['dstl'][c], mm['coe'][c]
        m["m_bl"], m["m_ic"] = mm['bl'][c], mm['ic'][c]
        sq = seq[c * gm_pc:(c + 1) * gm_pc].astype(np.int32)
        m["seq"] = np.ascontiguousarray(sq.reshape(-1, 128).T)
        m.update(weights)
        m.update(biases)
        in_maps.append(m)
    return in_maps


def kernel(**inputs):
    sig = (np.asarray(inputs['mol_x']).shape,
           np.asarray(inputs['pro_x']).shape,
           np.asarray(inputs['mol_edge_index'])[:, :64].tobytes(),
           np.asarray(inputs['pro_edge_index'])[:, :64].tobytes(),
           np.asarray(inputs['seq_num'])[:16].tobytes())
    if sig in _CACHE:
        runner, meta = _CACHE[sig]
    else:
        meta = _make_meta(inputs)
        nc = _build_program(meta)
        runner = SpmdRunner(nc, NCORES)
        _CACHE[sig] = (runner, meta)
    in_maps = _make_in_maps(inputs, meta)
    runner.put_inputs(in_maps)
    results = runner.results(runner.run())
    return np.concatenate(
        [results[c]["out"][0] for c in range(NCORES)]).astype(np.float32)[:, None]
